# revision 24
# baseline (speedup 1.0000x reference)
"""Trainium2 Bass kernel for nn_CPWGenerator (B=16384, D=128, P=10, F=1024).

Data-parallel over batch across 8 NeuronCores (2048 rows/core). Per core:
  - feature-major 3-layer MLPs (control-point head + weight head)
  - softmax denominator cancels: out = (basis @ (e*cpm)) / (basis @ e)
    with e = exp(logits) raw (scale-invariant; the reference's +1e-8 eps
    term shifts the result by <1.1e-7 of scale here — measured — because
    den >= 0.07)
  - final basis matmuls produce batch-major [128, F] output tiles directly
  - division: reciprocal_approx_fast (DVE) + multiply (DVE/GPSIMD split)
Matmuls run as float32r (fp32 storage, 11-bit-mantissa operand rounding,
exact fp32 accumulation) at full PE rate.
"""
import sys
if "/opt/trn_rl_repo" not in sys.path:
    sys.path.insert(0, "/opt/trn_rl_repo")

from contextlib import ExitStack

import numpy as np

import concourse.bacc as bacc
import concourse.mybir as mybir
import concourse.tile as tile
from concourse.bass_utils import run_bass_kernel_spmd

F32 = mybir.dt.float32
F32R = mybir.dt.float32r
AF = mybir.ActivationFunctionType

# problem shapes (hardcoded per contest contract)
B, D, P, F = 16384, 128, 10, 1024
NCORES = 8
BC = B // NCORES          # rows per core = 2048
BLOCKS = [512, 512, 512, 512]   # batch blocks (sum = BC)
EPS = 1e-8

# (block, j) pairs whose final multiply runs on GPSIMD (ACT copies the
# numerator out of PSUM first); the rest multiply on DVE straight from PSUM.
GP_MUL = {(0, 1), (0, 3), (1, 1), (1, 3),
          (2, 1), (2, 3), (3, 1), (3, 2), (3, 3)}
# (block, j) pairs whose output DMA issues from GPSIMD (SWDGE) instead of
# the SP HWDGE queue, to spread DMA issue across queues.
GP_DMA = set()

# f32r const blob column offsets
_C_W1T = 0            # [128 x 128]
_C_W2T = 128          # [128 x 256]
_C_W3T = 384          # [128 x 40]  (W3Ta | W3Tb, 20 cols each)
_C_WW1T = 424         # [128 x 64]
_C_WW2T = 488         # [64  x 128]
_C_WW3T = 616         # [128 x 10]
_C_P20 = 626          # [20  x 10]  pairing matrix (0.5 per pair)
C_R = 636

# fp32 const blob columns
_C_ID = 0             # [128 x 128] identity
_C_B1 = 128
_C_B2A = 129
_C_B2B = 130
_C_B3 = 131
_C_WB1 = 132
_C_WB2 = 133
_C_WB3 = 134
C_F = 135


def round_f32r(x: np.ndarray) -> np.ndarray:
    """fp32 -> fp32r rounding (keep 11 explicit mantissa bits, RNE).
    Matches TRN2 hardware exactly (validated on device)."""
    u = np.ascontiguousarray(x, dtype=np.float32).view(np.uint32)
    keep = np.uint32(0xFFFFF000)
    half = np.uint32(0x800)
    lsb = (u >> np.uint32(12)) & np.uint32(1)
    r = (u + half - np.uint32(1) + lsb) & keep
    return r.view(np.float32)


def basis_matrix() -> np.ndarray:
    """Replicates reference._basis_matrix in float32."""
    t = np.linspace(0.0, 1.0, F, dtype=np.float32)
    centers = (np.arange(P, dtype=np.float32) / np.float32(P - 1))
    sigma = np.float32(1.0 / P)
    z = (t[:, None] - centers[None, :]).astype(np.float32)
    basis = np.exp(-(z * z) / (np.float32(2.0) * sigma * sigma),
                   dtype=np.float32)
    return basis / (basis.sum(axis=1, keepdims=True) + np.float32(EPS))


def build_program():
    nc = bacc.Bacc()
    x_in = nc.declare_dram_parameter("x", [BC, D], F32, isOutput=False)
    wr_in = nc.declare_dram_parameter("wr", [128, C_R], F32R, isOutput=False)
    bt_in = nc.declare_dram_parameter("bt", [P, F], F32R, isOutput=False)
    wf_in = nc.declare_dram_parameter("wf", [128, C_F], F32, isOutput=False)
    out = nc.declare_dram_parameter("out", [BC, F], F32, isOutput=True)

    with tile.TileContext(nc) as tc, ExitStack() as ctx:
        cpool = ctx.enter_context(tc.tile_pool(name="const", bufs=1))
        wpool = ctx.enter_context(tc.tile_pool(name="work", bufs=2))
        npool = ctx.enter_context(tc.tile_pool(name="numcp", bufs=2))
        rpool = ctx.enter_context(tc.tile_pool(name="recip", bufs=2))
        opool = ctx.enter_context(tc.tile_pool(name="outp", bufs=4))
        ppool = ctx.enter_context(tc.tile_pool(name="psum", bufs=4, space="PSUM"))
        qpool = ctx.enter_context(tc.tile_pool(name="psumo", bufs=2, space="PSUM"))

        wr = cpool.tile([128, C_R], F32R)
        bt = cpool.tile([P, F], F32R)
        wf = cpool.tile([128, C_F], F32)
        xall = cpool.tile([128, BC], F32)

        def x_dma(xoff, nb_):
            nc.gpsimd.dma_start(
                xall[:, xoff:xoff + nb_].rearrange(
                    "p (c d) -> p c d", c=nb_ // 128),
                x_in[xoff:xoff + nb_, :].rearrange(
                    "(c p) d -> p c d", p=128),
            )

        # in-DMA order tuned for pipeline fill: identity+biases first (gates
        # the first transpose), then x block 0, then weights, then the rest
        nc.gpsimd.dma_start(wf[:], wf_in[:])
        x_dma(0, BLOCKS[0])
        nc.gpsimd.dma_start(wr[:], wr_in[:])
        nc.gpsimd.dma_start(bt[:], bt_in[:])
        xoff = BLOCKS[0]
        for nb_ in BLOCKS[1:]:
            x_dma(xoff, nb_)
            xoff += nb_

        ident = wf[:, _C_ID:_C_ID + 128]

        def mm(out_ap, lhsT, rhs, start=True, stop=True):
            nc.tensor.matmul(out_ap, lhsT, rhs, start=start, stop=stop)

        x0 = 0
        for blk, NB in enumerate(BLOCKS):

            # --- transpose x block: [128b,128d] chunks -> xT [128d, NB b]
            xtp = ppool.tile([128, NB], F32, tag="ps")
            for c in range(NB // 128):
                nc.tensor.matmul(
                    xtp[:, 128 * c:128 * (c + 1)],
                    xall[:, x0 + 128 * c:x0 + 128 * (c + 1)],
                    ident,
                    is_transpose=True,
                    start=(c % 4 == 0),
                    stop=(c % 4 == 3),
                )
            xt = wpool.tile([128, NB], F32R)
            nc.scalar.activation(xt[:], xtp[:], AF.Copy)

            # --- cp MLP (feature-major)
            h1p = ppool.tile([128, NB], F32, tag="ps")
            for n in range(NB // 512):
                mm(h1p[:, 512 * n:512 * (n + 1)],
                   wr[:, _C_W1T:_C_W1T + 128],
                   xt[:, 512 * n:512 * (n + 1)])
            h1 = wpool.tile([128, NB], F32R)
            nc.scalar.activation(h1[:], h1p[:], AF.Relu,
                                 bias=wf[:, _C_B1:_C_B1 + 1])

            h2pa = ppool.tile([128, NB], F32, tag="ps")
            for n in range(NB // 512):
                mm(h2pa[:, 512 * n:512 * (n + 1)],
                   wr[:, _C_W2T:_C_W2T + 128],
                   h1[:, 512 * n:512 * (n + 1)])
            h2a = wpool.tile([128, NB], F32R)
            nc.scalar.activation(h2a[:], h2pa[:], AF.Relu,
                                 bias=wf[:, _C_B2A:_C_B2A + 1])

            h2pb = ppool.tile([128, NB], F32, tag="ps")
            for n in range(NB // 512):
                mm(h2pb[:, 512 * n:512 * (n + 1)],
                   wr[:, _C_W2T + 128:_C_W2T + 256],
                   h1[:, 512 * n:512 * (n + 1)])
            h2b = wpool.tile([128, NB], F32R)
            nc.scalar.activation(h2b[:], h2pb[:], AF.Relu,
                                 bias=wf[:, _C_B2B:_C_B2B + 1])

            cpp = ppool.tile([20, NB], F32, tag="ps")
            for n in range(NB // 512):
                sl = slice(512 * n, 512 * (n + 1))
                mm(cpp[:, sl], wr[:, _C_W3T:_C_W3T + 20], h2a[:, sl],
                   stop=False)
                mm(cpp[:, sl], wr[:, _C_W3T + 20:_C_W3T + 40], h2b[:, sl],
                   start=False, stop=True)
            cp = wpool.tile([20, NB], F32R)
            nc.scalar.activation(cp[:], cpp[:], AF.Tanh,
                                 bias=wf[0:20, _C_B3:_C_B3 + 1])

            # --- w MLP
            g1p = ppool.tile([64, NB], F32, tag="ps")
            for n in range(NB // 512):
                mm(g1p[:, 512 * n:512 * (n + 1)],
                   wr[:, _C_WW1T:_C_WW1T + 64],
                   xt[:, 512 * n:512 * (n + 1)])
            g1 = wpool.tile([64, NB], F32R)
            nc.scalar.activation(g1[:], g1p[:], AF.Relu,
                                 bias=wf[0:64, _C_WB1:_C_WB1 + 1])

            g2p = ppool.tile([128, NB], F32, tag="ps")
            for n in range(NB // 512):
                mm(g2p[:, 512 * n:512 * (n + 1)],
                   wr[0:64, _C_WW2T:_C_WW2T + 128],
                   g1[:, 512 * n:512 * (n + 1)])
            g2 = wpool.tile([128, NB], F32R)
            nc.scalar.activation(g2[:], g2p[:], AF.Relu,
                                 bias=wf[:, _C_WB2:_C_WB2 + 1])

            wlp = ppool.tile([10, NB], F32, tag="ps")
            for n in range(NB // 512):
                mm(wlp[:, 512 * n:512 * (n + 1)],
                   wr[:, _C_WW3T:_C_WW3T + 10],
                   g2[:, 512 * n:512 * (n + 1)])
            e = wpool.tile([10, NB], F32R)
            nc.scalar.activation(e[:], wlp[:], AF.Exp,
                                 bias=wf[0:10, _C_WB3:_C_WB3 + 1])

            # --- pairing: cp_mean = P20.T @ cp -> [10, NB]
            pairp = ppool.tile([10, NB], F32, tag="ps")
            for n in range(NB // 512):
                sl = slice(512 * n, 512 * (n + 1))
                mm(pairp[:, sl], wr[0:20, _C_P20:_C_P20 + 10], cp[:, sl])

            # num lhsT rows: e * cp_mean  (DVE, psum x sbuf)
            wcpmN = wpool.tile([10, NB], F32R)
            nc.vector.tensor_mul(wcpmN[:], pairp[:], e[:].bitcast(F32))

            # --- output M-blocks (den emitted first so recip(j+1) can
            # overlap mul(j) with only 2 psum slots)
            for j in range(NB // 128):
                bsl = slice(128 * j, 128 * (j + 1))
                denp = qpool.tile([128, F], F32, tag="out")
                for h in range(F // 512):
                    fsl = slice(512 * h, 512 * (h + 1))
                    mm(denp[:, fsl], e[:, bsl], bt[:, fsl])
                nump = qpool.tile([128, F], F32, tag="out")
                for h in range(F // 512):
                    fsl = slice(512 * h, 512 * (h + 1))
                    mm(nump[:, fsl], wcpmN[:, bsl], bt[:, fsl])
                r = rpool.tile([128, F], F32)
                nc.vector.reciprocal_approx_fast(out=r[:], in_=denp[:])
                o = opool.tile([128, F], F32)
                if (blk, j) in GP_MUL:
                    numS = npool.tile([128, F], F32)
                    nc.scalar.copy(numS[:], nump[:])
                    nc.gpsimd.tensor_mul(o[:], numS[:], r[:])
                else:
                    nc.vector.tensor_mul(o[:], nump[:], r[:])
                dma_eng = nc.gpsimd if (blk, j) in GP_DMA else nc.sync
                dma_eng.dma_start(out[x0 + 128 * j:x0 + 128 * (j + 1), :],
                                  o[:])
            x0 += NB

    nc.compile()
    return nc


def host_consts(cp_w1, cp_b1, cp_w2, cp_b2, cp_w3, cp_b3,
                w_w1, w_b1, w_w2, w_b2, w_w3, w_b3):
    basis = basis_matrix()                     # [F, P]

    wr = np.zeros((128, C_R), np.float32)
    wr[:, _C_W1T:_C_W1T + 128] = cp_w1.T       # [128,128]
    wr[:, _C_W2T:_C_W2T + 256] = cp_w2.T       # [128,256]
    w3t = cp_w3.T                              # [256,20]
    wr[:, _C_W3T:_C_W3T + 20] = w3t[0:128]
    wr[:, _C_W3T + 20:_C_W3T + 40] = w3t[128:256]
    wr[:, _C_WW1T:_C_WW1T + 64] = w_w1.T       # [128,64]
    wr[0:64, _C_WW2T:_C_WW2T + 128] = w_w2.T   # [64,128]
    wr[:, _C_WW3T:_C_WW3T + 10] = w_w3.T       # [128,10]
    p20 = np.zeros((20, 10), np.float32)
    for p in range(P):
        p20[2 * p, p] = 0.5
        p20[2 * p + 1, p] = 0.5
    wr[0:20, _C_P20:_C_P20 + 10] = p20
    wr = round_f32r(wr)

    bt = round_f32r(np.ascontiguousarray(basis.T))   # [P, F]

    wf = np.zeros((128, C_F), np.float32)
    wf[:, _C_ID:_C_ID + 128] = np.eye(128, dtype=np.float32)
    wf[:, _C_B1] = cp_b1
    wf[:, _C_B2A] = cp_b2[0:128]
    wf[:, _C_B2B] = cp_b2[128:256]
    wf[0:20, _C_B3] = cp_b3
    wf[0:64, _C_WB1] = w_b1
    wf[:, _C_WB2] = w_b2
    wf[0:10, _C_WB3] = w_b3
    return wr, bt, wf


_NC_CACHE = None


def get_program():
    global _NC_CACHE
    if _NC_CACHE is None:
        _NC_CACHE = build_program()
    return _NC_CACHE


def kernel(x, cp_w1, cp_b1, cp_w2, cp_b2, cp_w3, cp_b3,
           w_w1, w_b1, w_w2, w_b2, w_w3, w_b3, _return_raw=False):
    x = np.asarray(x, np.float32)
    wr, bt, wf = host_consts(
        np.asarray(cp_w1, np.float32), np.asarray(cp_b1, np.float32),
        np.asarray(cp_w2, np.float32), np.asarray(cp_b2, np.float32),
        np.asarray(cp_w3, np.float32), np.asarray(cp_b3, np.float32),
        np.asarray(w_w1, np.float32), np.asarray(w_b1, np.float32),
        np.asarray(w_w2, np.float32), np.asarray(w_b2, np.float32),
        np.asarray(w_w3, np.float32), np.asarray(w_b3, np.float32))

    nc = get_program()
    in_maps = [
        {"x": np.ascontiguousarray(x[i * BC:(i + 1) * BC]),
         "wr": wr, "bt": bt, "wf": wf}
        for i in range(NCORES)
    ]
    res = run_bass_kernel_spmd(nc, in_maps, list(range(NCORES)))
    outs = [res.results[i]["out"] for i in range(NCORES)]
    full = np.concatenate(outs, axis=0)
    if _return_raw:
        return full, res
    return full
